# revision 41
# baseline (speedup 1.0000x reference)
"""A3C-LSTM-GA kernel for TRN2 (8 NeuronCores, SPMD-replicated).

Model: conv(8x8,s4) -> conv(4x4,s2) -> conv(4x4,s2) image backbone,
GRU(32->256) instruction encoder (18 sequential steps), 5-head gated
attention, linear(680->256), LSTMCell(256), critic/actor heads.

Strategy: batch-1 latency kernel; every core runs the identical program
(per the data-parallel sharding hint there is nothing to shard), output
taken from core 0. The 18-step GRU recurrence is the critical path; conv
work is scheduled by Tile into the PE/DVE idle slots of the GRU steps.

All matmuls run in bf16 (fp32 PSUM accumulation) except the tiny output
heads which stay fp32. All layout transforms / gathers / dtype casts of
constant inputs happen on the host inside kernel(); all FLOPs of the
model run on device.
"""

import numpy as np
import ml_dtypes

import concourse.bass as bass
import concourse.tile as tile
from concourse import bacc, mybir
from concourse.bass_utils import run_bass_kernel_spmd

BF = mybir.dt.bfloat16
F32 = mybir.dt.float32
AF = mybir.ActivationFunctionType
OP = mybir.AluOpType
BF_NP = ml_dtypes.bfloat16

N_CORES = 8

# ---- model dims (hardcoded; kernel.py must be self-contained) ----
IMG_H, IMG_W = 168, 300
OH1, OW1 = 41, 74      # conv1 out (stride 4, k8)
OH2, OW2 = 19, 36      # conv2 out (stride 2, k4)
OH3, OW3 = 8, 17       # conv3 out (stride 2, k4)
N1 = OH1 * OW1         # 3034
N2 = OH2 * OW2         # 684
N3 = OH3 * OW3         # 136
T = 18                 # instruction length
GH = 256               # GRU hidden

# blob layouts (element offsets into the packed DRAM parameters)
# p33 [33, 1042]: wordsb [33,18] | wih8 8x[33,128]
P33_W = 18 + 8 * 128
# gsm [128, 140]: ident [128,128] | hxc [128,2] | linb [128,2] | lb [128,8]
GSM_HXC = 128
GSM_LINB = GSM_HXC + 2
GSM_LB = GSM_LINB + 2
GSM_W = GSM_LB + 8
# whh [128, 1536]: whh12 12x[128,128]
# p64 [64, 4191]: x1b [64,3034] | w3 16x[64,64] | w1b [64,128] | abT [64,5]
P64_W3 = N1
P64_W1B = P64_W3 + 16 * 64
P64_ABT = P64_W1B + 128
P64_W = P64_ABT + 5
# biga [128, 1152]: w1a [128,128] | w2 16x[128,64]   (conv, needed early)
BIGA_W2 = 128
BIGA_W = BIGA_W2 + 16 * 64
# bigb [128, 7296]: aw10 10x[128,64] | linw 20x[128,128] | lw32 32x[128,128]
BIGB_LINW = 10 * 64
BIGB_LW = BIGB_LINW + 20 * 128
BIGB_W = BIGB_LW + 32 * 128
# f128 [128, 11] f32: cxc [128,2] | hdw [128,8] | b1 [128,1]
# f32s [32, 5] f32: temb [32,1] | hdw2 [32,4]
# f1  [1, 5] f32: onep [1,1] | hdb [1,4]

_CACHE = {}


def _build():
    """Build the Bass program once."""
    nc = bacc.Bacc("TRN2", target_bir_lowering=False, debug=False,
                   enable_asserts=False)

    d_p33 = nc.declare_dram_parameter("p33", [33, P33_W], BF, isOutput=False)
    d_gsm = nc.declare_dram_parameter("gsm", [128, GSM_W], BF, isOutput=False)
    d_whh = nc.declare_dram_parameter("whh", [128, 1536], BF, isOutput=False)
    d_f128 = nc.declare_dram_parameter("f128", [128, 11], F32, isOutput=False)
    d_x1a = nc.declare_dram_parameter("x1a", [128, N1], BF, isOutput=False)
    d_p64 = nc.declare_dram_parameter("p64", [64, P64_W], BF, isOutput=False)
    d_biga = nc.declare_dram_parameter("biga", [128, BIGA_W], BF, isOutput=False)
    d_bigb = nc.declare_dram_parameter("bigb", [128, BIGB_W], BF, isOutput=False)
    d_f32s = nc.declare_dram_parameter("f32s", [32, 5], F32, isOutput=False)
    d_f64 = nc.declare_dram_parameter("f64", [64, 2], F32, isOutput=False)
    d_f1 = nc.declare_dram_parameter("f1", [1, 5], F32, isOutput=False)
    d_out = nc.declare_dram_parameter("out", [516], F32, isOutput=True)

    with tile.TileContext(nc) as tc:
        with (
            tc.tile_pool(name="const", bufs=1) as cpool,
            tc.tile_pool(name="work", bufs=1) as wpool,
            tc.tile_pool(name="hpool", bufs=3) as hpool,
            tc.tile_pool(name="ew", bufs=2) as epool,
            tc.tile_pool(name="pstep", bufs=2, space="PSUM") as pstep,
            tc.tile_pool(name="pc1", bufs=2, space="PSUM") as pc1,
            tc.tile_pool(name="pc2", bufs=1, space="PSUM") as pc2,
            tc.tile_pool(name="ptail", bufs=1, space="PSUM") as ptail,
        ):
            # -------- input DMAs. GRU-critical blobs go FIRST on the sync
            # (HWDGE) queue in dependency order; only the big late-needed
            # weight blob rides gpsimd (SWDGE). The ACT engine issues no
            # DMAs so table loads + warmup run early.
            sp33 = cpool.tile([33, P33_W], BF)
            nc.sync.dma_start(out=sp33[:], in_=d_p33[:])
            sgsm = cpool.tile([128, GSM_W], BF)
            nc.sync.dma_start(out=sgsm[:], in_=d_gsm[:])
            swhh = cpool.tile([128, 1536], BF)
            nc.sync.dma_start(out=swhh[:], in_=d_whh[:])
            sx1a = cpool.tile([128, N1], BF)
            nc.sync.dma_start(out=sx1a[:], in_=d_x1a[:])
            sp64 = cpool.tile([64, P64_W], BF)
            nc.sync.dma_start(out=sp64[:], in_=d_p64[:])
            sf128 = cpool.tile([128, 11], F32)
            nc.sync.dma_start(out=sf128[:], in_=d_f128[:])
            sbiga = cpool.tile([128, BIGA_W], BF)
            nc.sync.dma_start(out=sbiga[:], in_=d_biga[:])
            sf64 = cpool.tile([64, 2], F32)
            nc.sync.dma_start(out=sf64[:], in_=d_f64[:])
            sf32s = cpool.tile([32, 5], F32)
            nc.sync.dma_start(out=sf32s[:], in_=d_f32s[:])
            sf1 = cpool.tile([1, 5], F32)
            nc.sync.dma_start(out=sf1[:], in_=d_f1[:])
            sbigb = cpool.tile([128, BIGB_W], BF)
            nc.sync.dma_start(out=sbigb[:], in_=d_bigb[:])

            ident = sgsm[:, 0:128]

            def whh_sl(kc, j):
                i = kc * 6 + j
                return swhh[:, i * 128:(i + 1) * 128]

            # ---------------- ACT table warmup ----------------------------
            # Trigger both table loads (sigmoid set, then the tanh set that
            # also holds sigmoid) during the DMA head, off the GRU path.
            warm = wpool.tile([1, 2], F32)
            nc.vector.memset(warm[:], 0.0)
            warm2 = wpool.tile([1, 2], F32)
            nc.scalar.activation(warm2[:], warm[:], AF.Sigmoid)
            nc.scalar.activation(warm2[:], warm[:], AF.Tanh)

            # ---------------- GRU input-side gates (GI) -------------------
            # psgi [128, 8, 18]; strip j: j<4 -> gi_rz + bih_rz + bhh_rz,
            # j in 4,5 -> bhh_g broadcast, j in 6,7 -> ig + bih_g
            psgi = pc2.tile([128, 8, T], F32, tag="c2")
            wordsb = sp33[:, 0:T]
            for j in range(8):
                wj = sp33[:, T + j * 128: T + (j + 1) * 128]
                nc.tensor.matmul(psgi[:, j, :], wj, wordsb,
                                 start=True, stop=True)
            gics = wpool.tile([128, 8, T], BF)
            nc.vector.tensor_copy(gics[:], psgi[:])

            # ---------------- GRU recurrence (critical path) --------------
            # rz gates and hg live in SEPARATE psum banks so the ACT sigmoid
            # read and the DVE hg evacuation don't serialize on one bank.
            h_prev = hpool.tile([128, 2], BF, tag="h")
            nc.vector.memset(h_prev[:], 0.0)

            for t in range(T):
                if t == 0:
                    # step 0: h=0, so the gates ARE the gi psum -- read it
                    # directly and skip the gics-copy dependency.
                    ps_rz = psgi[:, 0:4, 0]
                    ps_hg = psgi[:, 4:6, 0]
                else:
                    ps_rz_t = pstep.tile([128, 4], F32, tag="rz")
                    ps_hg_t = pstep.tile([128, 2], F32, tag="hg")
                    ps_rz = ps_rz_t[:]
                    ps_hg = ps_hg_t[:]
                    nc.tensor.matmul(ps_rz, ident, gics[:, 0:4, t],
                                     start=True, stop=False,
                                     skip_group_check=True)
                    for j in range(4):
                        for kc in range(2):
                            nc.tensor.matmul(
                                ps_rz[:, j:j + 1], whh_sl(kc, j),
                                h_prev[:, kc:kc + 1],
                                start=False, stop=(j == 3 and kc == 1),
                                skip_group_check=True)
                    nc.tensor.matmul(ps_hg, ident, gics[:, 4:6, t],
                                     start=True, stop=False,
                                     skip_group_check=True)
                    for j in (4, 5):
                        for kc in range(2):
                            nc.tensor.matmul(
                                ps_hg[:, j - 4:j - 3], whh_sl(kc, j),
                                h_prev[:, kc:kc + 1],
                                start=False, stop=(j == 5 and kc == 1),
                                skip_group_check=True)
                rz = epool.tile([128, 4], F32, tag="rz")
                nc.scalar.activation(rz[:], ps_rz, AF.Sigmoid)
                # off-critical-path (overlap the sigmoid):
                hgs = epool.tile([128, 2], F32, tag="hg")
                nc.vector.tensor_copy(hgs[:], ps_hg)
                # critical: u = r*hg ; v = u + ig ; g = tanh(v)
                ut = epool.tile([128, 2], F32, tag="u")
                nc.vector.tensor_mul(ut[:], rz[:, 0:2], hgs[:])
                vt = epool.tile([128, 2], F32, tag="v")
                nc.vector.tensor_add(vt[:], ut[:], gics[:, 6:8, t])
                gt = epool.tile([128, 2], F32, tag="g")
                nc.scalar.activation(gt[:], vt[:], AF.Tanh)
                # off-critical-path (overlap the tanh): p0 = z*h, w = 1-z
                p0 = epool.tile([128, 2], F32, tag="p0")
                nc.vector.tensor_mul(p0[:], rz[:, 2:4], h_prev[:])
                wt = epool.tile([128, 2], F32, tag="w")
                nc.vector.tensor_scalar(wt[:], rz[:, 2:4], -1.0, 1.0,
                                        OP.mult, OP.add)
                # critical tail: h' = g*w + p0
                r2 = epool.tile([128, 2], F32, tag="r2")
                nc.vector.tensor_mul(r2[:], gt[:], wt[:])
                h_new = hpool.tile([128, 2], BF, tag="h")
                nc.vector.tensor_add(h_new[:], r2[:], p0[:])
                h_prev = h_new

            # ---------------- conv1 --------------------------------------
            b1 = sf128[:, 10:11]
            h1 = wpool.tile([128, N1], BF)
            w1a = sbiga[:, 0:128]
            w1b = sp64[:, P64_W1B:P64_W1B + 128]
            x1b = sp64[:, 0:N1]
            # small matmul/relu chunks so conv work never blocks a GRU step's
            # PE/DVE ops for more than ~100-260 ns
            off = 0
            while off < N1:
                nw = min(256, N1 - off)
                ps1 = pc1.tile([128, nw], F32, tag="c1")
                nc.tensor.matmul(ps1[:], w1a, sx1a[:, off:off + nw],
                                 start=True, stop=False)
                nc.tensor.matmul(ps1[:], w1b[0:64, :], x1b[0:64, off:off + nw],
                                 start=False, stop=True)
                roff = 0
                while roff < nw:
                    rw = min(128, nw - roff)
                    nc.vector.tensor_scalar(
                        h1[:, off + roff:off + roff + rw],
                        ps1[:, roff:roff + rw], b1, 0.0, OP.add, OP.max)
                    roff += rw
                off += nw

            # ---------------- conv2 --------------------------------------
            h1v = h1[:].rearrange("p (h w) -> p h w", h=OH1, w=OW1)
            h2 = wpool.tile([64, N2], BF)
            h2v = h2[:].rearrange("p (h w) -> p h w", h=OH2, w=OW2)
            for half in range(2):
                ps2 = pc2.tile([64, OH2 * 18], F32, tag="c2")
                for c16 in range(16):
                    kh, kw = divmod(c16, 4)
                    rhs = h1v[:, kh:kh + 37:2,
                              kw + 36 * half:kw + 36 * half + 35:2]
                    nc.tensor.matmul(ps2[:], sbiga[:, BIGA_W2 + c16 * 64:
                                                    BIGA_W2 + (c16 + 1) * 64],
                                     rhs, start=(c16 == 0), stop=(c16 == 15))
                ps2v = ps2[:].rearrange("p (h w) -> p h w", h=OH2, w=18)
                for rh in range(2):
                    nc.vector.tensor_scalar(
                        h2v[:, :, 18 * half + 9 * rh:18 * half + 9 * rh + 9],
                        ps2v[:, :, 9 * rh:9 * rh + 9],
                        sf64[:, 0:1], 0.0, OP.add, OP.max)
            # ---------------- conv3 --------------------------------------
            img = wpool.tile([64, N3], BF)
            ps3 = ptail.tile([64, N3], F32, tag="tail")
            for c16 in range(16):
                kh, kw = divmod(c16, 4)
                rhs = h2v[:, kh:kh + 15:2, kw:kw + 33:2]
                nc.tensor.matmul(ps3[:], sp64[0:64, P64_W3 + c16 * 64:
                                              P64_W3 + (c16 + 1) * 64],
                                 rhs, start=(c16 == 0), stop=(c16 == 15))
            nc.vector.tensor_scalar(img[:], ps3[:],
                                    sf64[:, 1:2], 0.0, OP.add, OP.max)

            # ---------------- gated attention -----------------------------
            instr = h_prev  # [128, 2] bf16
            psA = ptail.tile([64, 5], F32, tag="tail")
            nc.tensor.matmul(psA[:], ident[0:64, 0:64],
                             sp64[0:64, P64_ABT:P64_ABT + 5],
                             start=True, stop=False, skip_group_check=True)
            for kk in range(5):
                for kc in range(2):
                    i = kk * 2 + kc
                    nc.tensor.matmul(psA[:, kk:kk + 1],
                                     sbigb[:, i * 64:(i + 1) * 64],
                                     instr[:, kc:kc + 1],
                                     start=False, stop=(kk == 4 and kc == 1),
                                     skip_group_check=True)
            attn = wpool.tile([64, 5], BF)
            nc.vector.tensor_scalar_max(attn[:], psA[:], 0.0)

            # attn_vecs TRANSPOSED directly: avT = img.T @ attn, so the
            # linear layer's lhsT columns come out of PSUM with no PE
            # transposes and no extra copies. (Reuses idle GRU psum slots.)
            psTa = pstep.tile([128, 5], F32, tag="rz")
            nc.tensor.matmul(psTa[:], img[:, 0:128], attn[:],
                             start=True, stop=True)
            psTb = pstep.tile([8, 5], F32, tag="hg")
            nc.tensor.matmul(psTb[:], img[:, 128:136], attn[:],
                             start=True, stop=True)
            avTa = wpool.tile([128, 5], BF)
            nc.vector.tensor_copy(avTa[:], psTa[:])
            avTb = wpool.tile([8, 5], BF)
            nc.vector.tensor_copy(avTb[:], psTb[:])

            # ---------------- linear 680 -> 256 ---------------------------
            psF = pc2.tile([128, 2], F32, tag="c2")
            nc.tensor.matmul(psF[:], ident, sgsm[:, GSM_LINB:GSM_LINB + 2],
                             start=True, stop=False, skip_group_check=True)
            for c in range(2):
                for kk in range(5):
                    i = (c * 5 + kk) * 2
                    nc.tensor.matmul(psF[:, c:c + 1],
                                     sbigb[:, BIGB_LINW + i * 128:
                                          BIGB_LINW + (i + 1) * 128],
                                     avTa[:, kk:kk + 1],
                                     start=False, stop=False,
                                     skip_group_check=True)
                    sl = sbigb[:, BIGB_LINW + (i + 1) * 128:
                               BIGB_LINW + (i + 2) * 128]
                    nc.tensor.matmul(psF[:, c:c + 1], sl[0:8, :],
                                     avTb[:, kk:kk + 1],
                                     start=False, stop=(kk == 4),
                                     skip_group_check=True)
            feat = wpool.tile([128, 2], BF)
            nc.vector.tensor_scalar_max(feat[:], psF[:], 0.0)

            # ---------------- LSTM cell -----------------------------------
            # bias + hx-side matmuls have no feat dependency -- listed first
            # so they run during the attention/linear phase.
            hxc = sgsm[:, GSM_HXC:GSM_HXC + 2]
            cxc = sf128[:, 0:2]
            psL = pc1.tile([128, 8], F32, tag="c1")
            nc.tensor.matmul(psL[:], ident, sgsm[:, GSM_LB:GSM_LB + 8],
                             start=True, stop=False, skip_group_check=True)
            for kc in (2, 3):
                for s in range(8):
                    i = s * 4 + kc
                    nc.tensor.matmul(psL[:, s:s + 1],
                                     sbigb[:, BIGB_LW + i * 128:
                                          BIGB_LW + (i + 1) * 128],
                                     hxc[:, kc - 2:kc - 1], start=False,
                                     stop=False, skip_group_check=True)
            for kc in (0, 1):
                for s in range(8):
                    i = s * 4 + kc
                    nc.tensor.matmul(psL[:, s:s + 1],
                                     sbigb[:, BIGB_LW + i * 128:
                                          BIGB_LW + (i + 1) * 128],
                                     feat[:, kc:kc + 1], start=False,
                                     stop=(s == 7 and kc == 1),
                                     skip_group_check=True)
            sg = epool.tile([128, 8], F32, tag="sg")
            nc.scalar.activation(sg[:], psL[:], AF.Sigmoid)
            tg = epool.tile([128, 2], F32, tag="tg")
            nc.scalar.activation(tg[:], psL[:, 4:6], AF.Tanh)
            # heads rhs folding: (hdw * sigma_o) so the head matmuls need
            # only tanh(c2), shaving the final h2 multiply off their path
            hdp0 = wpool.tile([128, 4], F32)
            nc.vector.tensor_scalar(hdp0[:], sf128[:, 2:6], sg[:, 6:7],
                                    None, OP.mult)
            hdp1 = wpool.tile([128, 4], F32)
            nc.vector.tensor_scalar(hdp1[:], sf128[:, 6:10], sg[:, 7:8],
                                    None, OP.mult)
            m1 = epool.tile([128, 2], F32, tag="m1")
            nc.vector.tensor_mul(m1[:], sg[:, 2:4], cxc)
            m2 = epool.tile([128, 2], F32, tag="m2")
            nc.vector.tensor_mul(m2[:], sg[:, 0:2], tg[:])
            hc2 = wpool.tile([128, 4], F32)   # cols 0:2 h2x, 2:4 c2
            c2o = hc2[:, 2:4]
            nc.vector.tensor_add(c2o, m1[:], m2[:])
            tc2 = epool.tile([128, 2], F32, tag="tc2")
            nc.scalar.activation(tc2[:], c2o, AF.Tanh)
            h2o = hc2[:, 0:2]
            nc.vector.tensor_mul(h2o, sg[:, 6:8], tc2[:])

            # ---------------- heads (fp32) --------------------------------
            psH = pc1.tile([1, 4], F32, tag="c1")
            nc.tensor.matmul(psH[:], sf1[:, 0:1], sf1[:, 1:5],
                             start=True, stop=False, skip_group_check=True)
            nc.tensor.matmul(psH[:], sf32s[:, 0:1], sf32s[:, 1:5],
                             start=False, stop=False, skip_group_check=True)
            nc.tensor.matmul(psH[:], tc2[:, 0:1], hdp0[:],
                             start=False, stop=False, skip_group_check=True)
            nc.tensor.matmul(psH[:], tc2[:, 1:2], hdp1[:],
                             start=False, stop=True, skip_group_check=True)

            oH = wpool.tile([1, 4], F32)
            nc.vector.tensor_copy(oH[:], psH[:])

            # ---------------- outputs (2 parallel DMAs) -------------------
            nc.gpsimd.dma_start(out=d_out[0:4], in_=oH[:])
            ohc = d_out[4:516].rearrange("(c p) -> p c", p=128)
            nc.sync.dma_start(out=ohc, in_=hc2[:])

    nc.compile()
    return nc


def _prep(inputs):
    """Host-side layout prep. Returns the in_map for one core."""
    f32 = np.float32
    g = lambda k: np.asarray(inputs[k], dtype=f32)
    x = g('x')[0]                         # [3, 168, 300]
    input_inst = np.asarray(inputs['input_inst']).astype(np.int64)[0]
    tx = int(np.asarray(inputs['tx']).reshape(-1)[0])

    # ---- conv1 im2col [192, 3034] ----
    sc, sh, sw = x.strides
    pat = np.lib.stride_tricks.as_strided(
        x, shape=(3, 8, 8, OH1, OW1), strides=(sc, sh, sw, 4 * sh, 4 * sw))
    X = pat.reshape(192, N1)
    x1a = np.ascontiguousarray(X[0:128]).astype(BF_NP)
    x1b = X[128:192].astype(BF_NP)        # [64, 3034]

    w1 = g('conv1_w').transpose(1, 2, 3, 0).reshape(192, 128)  # [k, m]
    w1a = w1[0:128].astype(BF_NP)
    w1b = w1[128:192].astype(BF_NP)       # [64, 128]
    w2 = g('conv2_w').transpose(2, 3, 1, 0).reshape(16, 128, 64).astype(BF_NP)
    w3 = g('conv3_w').transpose(2, 3, 1, 0).reshape(16, 64, 64).astype(BF_NP)

    # ---- GRU prep ----
    emb = g('emb')
    words = emb[input_inst]               # [18, 32]
    wordsb = np.ones((33, T), f32)
    wordsb[0:32] = words.T
    wih = g('gru_wih')                    # [768, 32]
    bih = g('gru_bih')
    bhh = g('gru_bhh')
    wih8 = np.zeros((8, 33, 128), f32)
    for j in range(4):                    # rz strips + both biases
        wih8[j, 0:32] = wih[j * 128:(j + 1) * 128].T
        wih8[j, 32] = bih[j * 128:(j + 1) * 128] + bhh[j * 128:(j + 1) * 128]
    for j in (4, 5):                      # bhh_g broadcast strips
        wih8[j, 32] = bhh[j * 128:(j + 1) * 128]
    for j in (6, 7):                      # true ig strips
        wih8[j, 0:32] = wih[(j - 2) * 128:(j - 1) * 128].T
        wih8[j, 32] = bih[(j - 2) * 128:(j - 1) * 128]
    p33 = np.zeros((33, P33_W), BF_NP)
    p33[:, 0:T] = wordsb.astype(BF_NP)
    p33[:, T:] = wih8.transpose(1, 0, 2).reshape(33, 8 * 128).astype(BF_NP)

    whh = g('gru_whh')                    # [768, 256]
    whh12 = whh.T.reshape(2, 128, 6, 128).transpose(0, 2, 1, 3)  # kc,j,k,m
    hx = g('hx').reshape(2, 128).T        # [128, 2]
    lin_b = g('lin_b').reshape(2, 128).T  # [128, 2]
    lstm_b = (g('lstm_bih') + g('lstm_bhh')).reshape(8, 128).T   # [128, 8]
    whhb = whh12.reshape(12, 128, 128).transpose(1, 0, 2).reshape(
        128, 12 * 128).astype(BF_NP)
    gsm = np.zeros((128, GSM_W), BF_NP)
    gsm[:, 0:128] = np.eye(128, dtype=BF_NP)
    gsm[:, GSM_HXC:GSM_HXC + 2] = hx.astype(BF_NP)
    gsm[:, GSM_LINB:GSM_LINB + 2] = lin_b.astype(BF_NP)
    gsm[:, GSM_LB:GSM_LB + 8] = lstm_b.astype(BF_NP)

    # ---- attention / linear / lstm weights ----
    aw = g('attn_w').transpose(0, 2, 1).reshape(5, 2, 128, 64)  # kk,kc,k,c
    aw10 = aw.reshape(10, 128, 64)
    abT = g('attn_b').T                   # [64, 5]
    lin_w = g('lin_w')                    # [256, 680]
    lw = lin_w.reshape(2, 128, 5, 136).transpose(0, 2, 3, 1)  # c,kk,hw,m
    linw20 = np.zeros((2, 5, 2, 128, 128), f32)
    linw20[:, :, 0] = lw[:, :, 0:128]
    linw20[:, :, 1, 0:8] = lw[:, :, 128:136]
    linw20 = linw20.reshape(20, 128, 128)
    A = g('lstm_wih').T                   # [256, 1024]
    B = g('lstm_whh').T
    lw32 = np.zeros((8, 4, 128, 128), f32)
    for s in range(8):
        for kc in range(4):
            src = A if kc < 2 else B
            k0 = (kc % 2) * 128
            lw32[s, kc] = src[k0:k0 + 128, s * 128:(s + 1) * 128]
    lw32 = lw32.reshape(32, 128, 128)

    biga = np.zeros((128, BIGA_W), BF_NP)
    biga[:, 0:128] = w1a
    biga[:, BIGA_W2:] = w2.transpose(1, 0, 2).reshape(128, 16 * 64)
    bigb = np.zeros((128, BIGB_W), BF_NP)
    bigb[:, 0:BIGB_LINW] = aw10.transpose(1, 0, 2).reshape(
        128, 10 * 64).astype(BF_NP)
    bigb[:, BIGB_LINW:BIGB_LW] = linw20.transpose(1, 0, 2).reshape(
        128, 20 * 128).astype(BF_NP)
    bigb[:, BIGB_LW:] = lw32.transpose(1, 0, 2).reshape(
        128, 32 * 128).astype(BF_NP)

    p64 = np.zeros((64, P64_W), BF_NP)
    p64[:, 0:N1] = x1b
    p64[:, P64_W3:P64_W1B] = w3.transpose(1, 0, 2).reshape(64, 16 * 64)
    p64[:, P64_W1B:P64_W1B + 128] = w1b
    p64[:, P64_ABT:] = abT.astype(BF_NP)

    # ---- f32 blobs ----
    f128 = np.zeros((128, 11), f32)
    f128[:, 0:2] = g('cx').reshape(2, 128).T
    headW = np.concatenate([g('critic_w'), g('actor_w')], axis=0).T  # [288,4]
    f128[:, 2:6] = headW[0:128]
    f128[:, 6:10] = headW[128:256]
    f128[:, 10] = g('conv1_b')
    f64 = np.zeros((64, 2), f32)
    f64[:, 0] = g('conv2_b')
    f64[:, 1] = g('conv3_b')
    f32s = np.zeros((32, 5), f32)
    f32s[:, 0] = g('time_emb')[tx]
    f32s[:, 1:5] = headW[256:288]
    f1 = np.zeros((1, 5), f32)
    f1[0, 0] = 1.0
    f1[0, 1] = g('critic_b').reshape(-1)[0]
    f1[0, 2:5] = g('actor_b').reshape(-1)
    return {
        "p33": p33, "gsm": gsm, "whh": whhb, "f128": f128, "x1a": x1a,
        "p64": p64, "biga": biga, "bigb": bigb, "f32s": f32s, "f64": f64,
        "f1": f1,
    }


def kernel(**inputs):
    if "nc" not in _CACHE:
        _CACHE["nc"] = _build()
    nc = _CACHE["nc"]
    in_map = _prep(inputs)
    in_maps = [in_map for _ in range(N_CORES)]
    res = run_bass_kernel_spmd(nc, in_maps, list(range(N_CORES)),
                               trace=_CACHE.get("trace", False))
    _CACHE["last"] = res
    out = np.asarray(res.results[0]["out"], dtype=np.float32)
    critic = out[0:1].reshape(1, 1)
    actor = out[1:4].reshape(1, 3)
    h2x = out[4:260].reshape(1, 256)
    c2 = out[260:516].reshape(1, 256)
    return critic, actor, h2x, c2


# revision 52
# speedup vs baseline: 1.0420x; 1.0420x over previous
"""A3C-LSTM-GA kernel for TRN2 (8 NeuronCores, SPMD-replicated).

Model: conv(8x8,s4) -> conv(4x4,s2) -> conv(4x4,s2) image backbone,
GRU(32->256) instruction encoder (18 sequential steps), 5-head gated
attention, linear(680->256), LSTMCell(256), critic/actor heads.

Strategy: batch-1 latency kernel; every core runs the identical program
(per the data-parallel sharding hint there is nothing to shard), output
taken from core 0. The 18-step GRU recurrence is the critical path; conv
work is scheduled by Tile into the PE/DVE idle slots of the GRU steps.

All matmuls run in bf16 (fp32 PSUM accumulation) except the tiny output
heads which stay fp32. All layout transforms / gathers / dtype casts of
constant inputs happen on the host inside kernel(); all FLOPs of the
model run on device.
"""

import numpy as np
import ml_dtypes

import concourse.bass as bass
import concourse.tile as tile
from concourse import bacc, mybir
from concourse.bass_utils import run_bass_kernel_spmd

BF = mybir.dt.bfloat16
F32 = mybir.dt.float32
AF = mybir.ActivationFunctionType
OP = mybir.AluOpType
BF_NP = ml_dtypes.bfloat16

N_CORES = 8

# ---- model dims (hardcoded; kernel.py must be self-contained) ----
IMG_H, IMG_W = 168, 300
OH1, OW1 = 41, 74      # conv1 out (stride 4, k8)
OH2, OW2 = 19, 36      # conv2 out (stride 2, k4)
OH3, OW3 = 8, 17       # conv3 out (stride 2, k4)
N1 = OH1 * OW1         # 3034
N2 = OH2 * OW2         # 684
N3 = OH3 * OW3         # 136
T = 18                 # instruction length
GH = 256               # GRU hidden

# blob layouts (element offsets into the packed DRAM parameters)
# p33 [33, 1042]: wordsb [33,18] | wih8 8x[33,128]
P33_W = 18 + 8 * 128
# gsm [128, 140]: ident [128,128] | hxc [128,2] | linb [128,2] | lb [128,8]
GSM_HXC = 128
GSM_LINB = GSM_HXC + 2
GSM_LB = GSM_LINB + 2
GSM_W = GSM_LB + 8
# whh [128, 1536]: whh12 12x[128,128]
# p64 [64, 4191]: x1b [64,3034] | w3 16x[64,64] | w1b [64,128] | abT [64,5]
P64_W3 = N1
P64_W1B = P64_W3 + 16 * 64
P64_ABT = P64_W1B + 128
P64_W = P64_ABT + 5
# biga [128, 1152]: w1a [128,128] | w2 16x[128,64]   (conv, needed early)
BIGA_W2 = 128
BIGA_W = BIGA_W2 + 16 * 64
# bigb [128, 7296]: aw10 10x[128,64] | linw 20x[128,128] | lw32 32x[128,128]
BIGB_LINW = 10 * 64
BIGB_LW = BIGB_LINW + 20 * 128
BIGB_W = BIGB_LW + 32 * 128
# f128 [128, 11] f32: cxc [128,2] | hdw [128,8] | b1 [128,1]
# f32s [32, 5] f32: temb [32,1] | hdw2 [32,4]
# f1  [1, 5] f32: onep [1,1] | hdb [1,4]

_CACHE = {}


def _build():
    """Build the Bass program once."""
    nc = bacc.Bacc("TRN2", target_bir_lowering=False, debug=False,
                   enable_asserts=False)

    d_p33 = nc.declare_dram_parameter("p33", [33, P33_W], BF, isOutput=False)
    d_gsm = nc.declare_dram_parameter("gsm", [128, GSM_W], BF, isOutput=False)
    d_whh = nc.declare_dram_parameter("whh", [128, 1536], BF, isOutput=False)
    d_f128 = nc.declare_dram_parameter("f128", [128, 11], F32, isOutput=False)
    d_x1a = nc.declare_dram_parameter("x1a", [128, N1], BF, isOutput=False)
    d_p64 = nc.declare_dram_parameter("p64", [64, P64_W], BF, isOutput=False)
    d_biga = nc.declare_dram_parameter("biga", [128, BIGA_W], BF, isOutput=False)
    d_bigb = nc.declare_dram_parameter("bigb", [128, BIGB_W], BF, isOutput=False)
    d_f32s = nc.declare_dram_parameter("f32s", [32, 5], F32, isOutput=False)
    d_f64 = nc.declare_dram_parameter("f64", [64, 2], F32, isOutput=False)
    d_f1 = nc.declare_dram_parameter("f1", [1, 5], F32, isOutput=False)
    d_out = nc.declare_dram_parameter("out", [516], F32, isOutput=True)

    with tile.TileContext(nc) as tc:
        with (
            tc.tile_pool(name="const", bufs=1) as cpool,
            tc.tile_pool(name="work", bufs=1) as wpool,
            tc.tile_pool(name="hpool", bufs=20) as hpool,
            tc.tile_pool(name="ew", bufs=20) as epool,
            tc.tile_pool(name="pstep", bufs=2, space="PSUM") as pstep,
            tc.tile_pool(name="pc1", bufs=2, space="PSUM") as pc1,
            tc.tile_pool(name="pc2", bufs=1, space="PSUM") as pc2,
            tc.tile_pool(name="ptail", bufs=1, space="PSUM") as ptail,
        ):
            # -------- input DMAs. GRU-critical blobs go FIRST on the sync
            # (HWDGE) queue in dependency order; only the big late-needed
            # weight blob rides gpsimd (SWDGE). The ACT engine issues no
            # DMAs so table loads + warmup run early.
            sp33 = cpool.tile([33, P33_W], BF)
            nc.sync.dma_start(out=sp33[:], in_=d_p33[:])
            sgsm = cpool.tile([128, GSM_W], BF)
            nc.sync.dma_start(out=sgsm[:], in_=d_gsm[:])
            swhh = cpool.tile([128, 1536], BF)
            nc.sync.dma_start(out=swhh[:], in_=d_whh[:])
            sx1a = cpool.tile([128, N1], BF)
            nc.sync.dma_start(out=sx1a[:], in_=d_x1a[:])
            sp64 = cpool.tile([64, P64_W], BF)
            nc.sync.dma_start(out=sp64[:], in_=d_p64[:])
            sf128 = cpool.tile([128, 11], F32)
            nc.sync.dma_start(out=sf128[:], in_=d_f128[:])
            sbiga = cpool.tile([128, BIGA_W], BF)
            nc.sync.dma_start(out=sbiga[:], in_=d_biga[:])
            sf64 = cpool.tile([64, 2], F32)
            nc.sync.dma_start(out=sf64[:], in_=d_f64[:])
            sf32s = cpool.tile([32, 5], F32)
            nc.sync.dma_start(out=sf32s[:], in_=d_f32s[:])
            sf1 = cpool.tile([1, 5], F32)
            nc.sync.dma_start(out=sf1[:], in_=d_f1[:])
            sbigb = cpool.tile([128, BIGB_W], BF)
            nc.sync.dma_start(out=sbigb[:], in_=d_bigb[:])

            ident = sgsm[:, 0:128]

            def whh_sl(kc, j):
                i = kc * 6 + j
                return swhh[:, i * 128:(i + 1) * 128]

            # ---------------- ACT table warmup ----------------------------
            # Trigger both table loads (sigmoid set, then the tanh set that
            # also holds sigmoid) during the DMA head, off the GRU path.
            warm = wpool.tile([1, 2], F32)
            nc.vector.memset(warm[:], 0.0)
            warm2 = wpool.tile([1, 2], F32)
            nc.scalar.activation(warm2[:], warm[:], AF.Sigmoid)
            nc.scalar.activation(warm2[:], warm[:], AF.Tanh)

            # ---------------- GRU input-side gates (GI) -------------------
            # psgi [128, 8, 18]; strip j: j<4 -> gi_rz + bih_rz + bhh_rz,
            # j in 4,5 -> bhh_g broadcast, j in 6,7 -> ig + bih_g
            psgi = pc2.tile([128, 8, T], F32, tag="c2")
            wordsb = sp33[:, 0:T]
            for j in range(8):
                wj = sp33[:, T + j * 128: T + (j + 1) * 128]
                nc.tensor.matmul(psgi[:, j, :], wj, wordsb,
                                 start=True, stop=True)
            gics = wpool.tile([128, 8, T], BF)
            nc.vector.tensor_copy(gics[:], psgi[:])

            # ---------------- GRU recurrence (critical path) --------------
            # rz gates and hg live in SEPARATE psum banks so the ACT sigmoid
            # read and the DVE hg evacuation don't serialize on one bank.
            h_prev = hpool.tile([128, 2], BF, tag="h")
            nc.vector.memset(h_prev[:], 0.0)

            for t in range(T):
                if t == 0:
                    # step 0: h=0, so the gates ARE the gi psum -- read it
                    # directly and skip the gics-copy dependency.
                    ps_rz = psgi[:, 0:4, 0]
                    ps_hg = psgi[:, 4:6, 0]
                else:
                    ps_rz_t = pstep.tile([128, 4], F32, tag="rz")
                    ps_hg_t = pstep.tile([128, 2], F32, tag="hg")
                    ps_rz = ps_rz_t[:]
                    ps_hg = ps_hg_t[:]
                    nc.tensor.matmul(ps_rz, ident, gics[:, 0:4, t],
                                     start=True, stop=False,
                                     skip_group_check=True)
                    for j in range(4):
                        for kc in range(2):
                            nc.tensor.matmul(
                                ps_rz[:, j:j + 1], whh_sl(kc, j),
                                h_prev[:, kc:kc + 1],
                                start=False, stop=(j == 3 and kc == 1),
                                skip_group_check=True)
                    nc.tensor.matmul(ps_hg, ident, gics[:, 4:6, t],
                                     start=True, stop=False,
                                     skip_group_check=True)
                    for j in (4, 5):
                        for kc in range(2):
                            nc.tensor.matmul(
                                ps_hg[:, j - 4:j - 3], whh_sl(kc, j),
                                h_prev[:, kc:kc + 1],
                                start=False, stop=(j == 5 and kc == 1),
                                skip_group_check=True)
                rz = epool.tile([128, 4], F32, tag="rz")
                nc.scalar.activation(rz[:], ps_rz, AF.Sigmoid)
                # off-critical-path (overlap the sigmoid):
                hgs = epool.tile([128, 2], F32, tag="hg")
                nc.vector.tensor_copy(hgs[:], ps_hg)
                # critical: u = r*hg ; v = u + ig ; g = tanh(v)
                ut = epool.tile([128, 2], F32, tag="u")
                nc.vector.tensor_mul(ut[:], rz[:, 0:2], hgs[:])
                vt = epool.tile([128, 2], F32, tag="v")
                nc.vector.tensor_add(vt[:], ut[:], gics[:, 6:8, t])
                gt = epool.tile([128, 2], F32, tag="g")
                nc.scalar.activation(gt[:], vt[:], AF.Tanh)
                # off-critical-path (overlap the tanh): p0 = z*h, w = 1-z
                p0 = epool.tile([128, 2], F32, tag="p0")
                nc.vector.tensor_mul(p0[:], rz[:, 2:4], h_prev[:])
                wt = epool.tile([128, 2], F32, tag="w")
                nc.vector.tensor_scalar(wt[:], rz[:, 2:4], -1.0, 1.0,
                                        OP.mult, OP.add)
                # critical tail: h' = g*w + p0
                r2 = epool.tile([128, 2], F32, tag="r2")
                nc.vector.tensor_mul(r2[:], gt[:], wt[:])
                h_new = hpool.tile([128, 2], BF, tag="h")
                nc.vector.tensor_add(h_new[:], r2[:], p0[:])
                h_prev = h_new

            # ---------------- conv1 --------------------------------------
            b1 = sf128[:, 10:11]
            h1 = wpool.tile([128, N1], BF)
            w1a = sbiga[:, 0:128]
            w1b = sp64[:, P64_W1B:P64_W1B + 128]
            x1b = sp64[:, 0:N1]
            # small matmul/relu chunks so conv work never blocks a GRU step's
            # PE/DVE ops for more than ~100-260 ns
            off = 0
            while off < N1:
                nw = min(256, N1 - off)
                ps1 = pc1.tile([128, nw], F32, tag="c1")
                nc.tensor.matmul(ps1[:], w1a, sx1a[:, off:off + nw],
                                 start=True, stop=False)
                nc.tensor.matmul(ps1[:], w1b[0:64, :], x1b[0:64, off:off + nw],
                                 start=False, stop=True)
                roff = 0
                while roff < nw:
                    rw = min(128, nw - roff)
                    nc.vector.tensor_scalar(
                        h1[:, off + roff:off + roff + rw],
                        ps1[:, roff:roff + rw], b1, 0.0, OP.add, OP.max)
                    roff += rw
                off += nw

            # ---------------- conv2 --------------------------------------
            h1v = h1[:].rearrange("p (h w) -> p h w", h=OH1, w=OW1)
            h2 = wpool.tile([64, N2], BF)
            h2v = h2[:].rearrange("p (h w) -> p h w", h=OH2, w=OW2)
            for half in range(2):
                ps2 = pc2.tile([64, OH2 * 18], F32, tag="c2")
                for c16 in range(16):
                    kh, kw = divmod(c16, 4)
                    rhs = h1v[:, kh:kh + 37:2,
                              kw + 36 * half:kw + 36 * half + 35:2]
                    nc.tensor.matmul(ps2[:], sbiga[:, BIGA_W2 + c16 * 64:
                                                    BIGA_W2 + (c16 + 1) * 64],
                                     rhs, start=(c16 == 0), stop=(c16 == 15))
                ps2v = ps2[:].rearrange("p (h w) -> p h w", h=OH2, w=18)
                for rh in range(2):
                    nc.vector.tensor_scalar(
                        h2v[:, :, 18 * half + 9 * rh:18 * half + 9 * rh + 9],
                        ps2v[:, :, 9 * rh:9 * rh + 9],
                        sf64[:, 0:1], 0.0, OP.add, OP.max)
            # ---------------- conv3 --------------------------------------
            img = wpool.tile([64, N3], BF)
            ps3 = ptail.tile([64, N3], F32, tag="tail")
            for c16 in range(16):
                kh, kw = divmod(c16, 4)
                rhs = h2v[:, kh:kh + 15:2, kw:kw + 33:2]
                nc.tensor.matmul(ps3[:], sp64[0:64, P64_W3 + c16 * 64:
                                              P64_W3 + (c16 + 1) * 64],
                                 rhs, start=(c16 == 0), stop=(c16 == 15))
            nc.vector.tensor_scalar(img[:], ps3[:],
                                    sf64[:, 1:2], 0.0, OP.add, OP.max)

            # ---------------- gated attention -----------------------------
            instr = h_prev  # [128, 2] bf16
            psA = ptail.tile([64, 5], F32, tag="tail")
            nc.tensor.matmul(psA[:], ident[0:64, 0:64],
                             sp64[0:64, P64_ABT:P64_ABT + 5],
                             start=True, stop=False, skip_group_check=True)
            for kk in range(5):
                for kc in range(2):
                    i = kk * 2 + kc
                    nc.tensor.matmul(psA[:, kk:kk + 1],
                                     sbigb[:, i * 64:(i + 1) * 64],
                                     instr[:, kc:kc + 1],
                                     start=False, stop=(kk == 4 and kc == 1),
                                     skip_group_check=True)
            attn = wpool.tile([64, 5], BF)
            nc.vector.tensor_scalar_max(attn[:], psA[:], 0.0)

            # attn_vecs TRANSPOSED directly: avT = img.T @ attn, so the
            # linear layer's lhsT columns come out of PSUM with no PE
            # transposes and no extra copies. (Reuses idle GRU psum slots.)
            psTa = pstep.tile([128, 5], F32, tag="rz")
            nc.tensor.matmul(psTa[:], img[:, 0:128], attn[:],
                             start=True, stop=True)
            psTb = pstep.tile([8, 5], F32, tag="hg")
            nc.tensor.matmul(psTb[:], img[:, 128:136], attn[:],
                             start=True, stop=True)
            avTa = wpool.tile([128, 5], BF)
            nc.vector.tensor_copy(avTa[:], psTa[:])
            avTb = wpool.tile([8, 5], BF)
            nc.vector.tensor_copy(avTb[:], psTb[:])

            # ---------------- linear 680 -> 256 ---------------------------
            psF = pc2.tile([128, 2], F32, tag="c2")
            nc.tensor.matmul(psF[:], ident, sgsm[:, GSM_LINB:GSM_LINB + 2],
                             start=True, stop=False, skip_group_check=True)
            for c in range(2):
                for kk in range(5):
                    i = (c * 5 + kk) * 2
                    nc.tensor.matmul(psF[:, c:c + 1],
                                     sbigb[:, BIGB_LINW + i * 128:
                                          BIGB_LINW + (i + 1) * 128],
                                     avTa[:, kk:kk + 1],
                                     start=False, stop=False,
                                     skip_group_check=True)
                    sl = sbigb[:, BIGB_LINW + (i + 1) * 128:
                               BIGB_LINW + (i + 2) * 128]
                    nc.tensor.matmul(psF[:, c:c + 1], sl[0:8, :],
                                     avTb[:, kk:kk + 1],
                                     start=False, stop=(kk == 4),
                                     skip_group_check=True)
            feat = wpool.tile([128, 2], BF)
            nc.vector.tensor_scalar_max(feat[:], psF[:], 0.0)

            # ---------------- LSTM cell -----------------------------------
            # bias + hx-side matmuls have no feat dependency -- listed first
            # so they run during the attention/linear phase.
            hxc = sgsm[:, GSM_HXC:GSM_HXC + 2]
            cxc = sf128[:, 0:2]
            psL = pc1.tile([128, 8], F32, tag="c1")
            nc.tensor.matmul(psL[:], ident, sgsm[:, GSM_LB:GSM_LB + 8],
                             start=True, stop=False, skip_group_check=True)
            for kc in (2, 3):
                for s in range(8):
                    i = s * 4 + kc
                    nc.tensor.matmul(psL[:, s:s + 1],
                                     sbigb[:, BIGB_LW + i * 128:
                                          BIGB_LW + (i + 1) * 128],
                                     hxc[:, kc - 2:kc - 1], start=False,
                                     stop=False, skip_group_check=True)
            for kc in (0, 1):
                for s in range(8):
                    i = s * 4 + kc
                    nc.tensor.matmul(psL[:, s:s + 1],
                                     sbigb[:, BIGB_LW + i * 128:
                                          BIGB_LW + (i + 1) * 128],
                                     feat[:, kc:kc + 1], start=False,
                                     stop=(s == 7 and kc == 1),
                                     skip_group_check=True)
            sg = epool.tile([128, 8], F32, tag="sg")
            nc.scalar.activation(sg[:], psL[:], AF.Sigmoid)
            tg = epool.tile([128, 2], F32, tag="tg")
            nc.scalar.activation(tg[:], psL[:, 4:6], AF.Tanh)
            # heads rhs folding: (hdw * sigma_o) so the head matmuls need
            # only tanh(c2), shaving the final h2 multiply off their path
            hdp0 = wpool.tile([128, 4], F32)
            nc.vector.tensor_scalar(hdp0[:], sf128[:, 2:6], sg[:, 6:7],
                                    None, OP.mult)
            hdp1 = wpool.tile([128, 4], F32)
            nc.vector.tensor_scalar(hdp1[:], sf128[:, 6:10], sg[:, 7:8],
                                    None, OP.mult)
            m1 = epool.tile([128, 2], F32, tag="m1")
            nc.vector.tensor_mul(m1[:], sg[:, 2:4], cxc)
            m2 = epool.tile([128, 2], F32, tag="m2")
            nc.vector.tensor_mul(m2[:], sg[:, 0:2], tg[:])
            hc2 = wpool.tile([128, 4], F32)   # cols 0:2 h2x, 2:4 c2
            c2o = hc2[:, 2:4]
            nc.vector.tensor_add(c2o, m1[:], m2[:])
            tc2 = epool.tile([128, 2], F32, tag="tc2")
            nc.scalar.activation(tc2[:], c2o, AF.Tanh)
            h2o = hc2[:, 0:2]
            nc.vector.tensor_mul(h2o, sg[:, 6:8], tc2[:])

            # ---------------- heads (fp32) --------------------------------
            psH = pc1.tile([1, 4], F32, tag="c1")
            nc.tensor.matmul(psH[:], sf1[:, 0:1], sf1[:, 1:5],
                             start=True, stop=False, skip_group_check=True)
            nc.tensor.matmul(psH[:], sf32s[:, 0:1], sf32s[:, 1:5],
                             start=False, stop=False, skip_group_check=True)
            nc.tensor.matmul(psH[:], tc2[:, 0:1], hdp0[:],
                             start=False, stop=False, skip_group_check=True)
            nc.tensor.matmul(psH[:], tc2[:, 1:2], hdp1[:],
                             start=False, stop=True, skip_group_check=True)

            oH = wpool.tile([1, 4], F32)
            nc.vector.tensor_copy(oH[:], psH[:])

            # ---------------- outputs (2 parallel DMAs) -------------------
            nc.gpsimd.dma_start(out=d_out[0:4], in_=oH[:])
            ohc = d_out[4:516].rearrange("(c p) -> p c", p=128)
            nc.sync.dma_start(out=ohc, in_=hc2[:])

    nc.compile()
    return nc


def _prep(inputs):
    """Host-side layout prep. Returns the in_map for one core."""
    f32 = np.float32
    g = lambda k: np.asarray(inputs[k], dtype=f32)
    x = g('x')[0]                         # [3, 168, 300]
    input_inst = np.asarray(inputs['input_inst']).astype(np.int64)[0]
    tx = int(np.asarray(inputs['tx']).reshape(-1)[0])

    # ---- conv1 im2col [192, 3034] ----
    sc, sh, sw = x.strides
    pat = np.lib.stride_tricks.as_strided(
        x, shape=(3, 8, 8, OH1, OW1), strides=(sc, sh, sw, 4 * sh, 4 * sw))
    X = pat.reshape(192, N1)
    x1a = np.ascontiguousarray(X[0:128]).astype(BF_NP)
    x1b = X[128:192].astype(BF_NP)        # [64, 3034]

    w1 = g('conv1_w').transpose(1, 2, 3, 0).reshape(192, 128)  # [k, m]
    w1a = w1[0:128].astype(BF_NP)
    w1b = w1[128:192].astype(BF_NP)       # [64, 128]
    w2 = g('conv2_w').transpose(2, 3, 1, 0).reshape(16, 128, 64).astype(BF_NP)
    w3 = g('conv3_w').transpose(2, 3, 1, 0).reshape(16, 64, 64).astype(BF_NP)

    # ---- GRU prep ----
    emb = g('emb')
    words = emb[input_inst]               # [18, 32]
    wordsb = np.ones((33, T), f32)
    wordsb[0:32] = words.T
    wih = g('gru_wih')                    # [768, 32]
    bih = g('gru_bih')
    bhh = g('gru_bhh')
    wih8 = np.zeros((8, 33, 128), f32)
    for j in range(4):                    # rz strips + both biases
        wih8[j, 0:32] = wih[j * 128:(j + 1) * 128].T
        wih8[j, 32] = bih[j * 128:(j + 1) * 128] + bhh[j * 128:(j + 1) * 128]
    for j in (4, 5):                      # bhh_g broadcast strips
        wih8[j, 32] = bhh[j * 128:(j + 1) * 128]
    for j in (6, 7):                      # true ig strips
        wih8[j, 0:32] = wih[(j - 2) * 128:(j - 1) * 128].T
        wih8[j, 32] = bih[(j - 2) * 128:(j - 1) * 128]
    p33 = np.zeros((33, P33_W), BF_NP)
    p33[:, 0:T] = wordsb.astype(BF_NP)
    p33[:, T:] = wih8.transpose(1, 0, 2).reshape(33, 8 * 128).astype(BF_NP)

    whh = g('gru_whh')                    # [768, 256]
    whh12 = whh.T.reshape(2, 128, 6, 128).transpose(0, 2, 1, 3)  # kc,j,k,m
    hx = g('hx').reshape(2, 128).T        # [128, 2]
    lin_b = g('lin_b').reshape(2, 128).T  # [128, 2]
    lstm_b = (g('lstm_bih') + g('lstm_bhh')).reshape(8, 128).T   # [128, 8]
    whhb = whh12.reshape(12, 128, 128).transpose(1, 0, 2).reshape(
        128, 12 * 128).astype(BF_NP)
    gsm = np.zeros((128, GSM_W), BF_NP)
    gsm[:, 0:128] = np.eye(128, dtype=BF_NP)
    gsm[:, GSM_HXC:GSM_HXC + 2] = hx.astype(BF_NP)
    gsm[:, GSM_LINB:GSM_LINB + 2] = lin_b.astype(BF_NP)
    gsm[:, GSM_LB:GSM_LB + 8] = lstm_b.astype(BF_NP)

    # ---- attention / linear / lstm weights ----
    aw = g('attn_w').transpose(0, 2, 1).reshape(5, 2, 128, 64)  # kk,kc,k,c
    aw10 = aw.reshape(10, 128, 64)
    abT = g('attn_b').T                   # [64, 5]
    lin_w = g('lin_w')                    # [256, 680]
    lw = lin_w.reshape(2, 128, 5, 136).transpose(0, 2, 3, 1)  # c,kk,hw,m
    linw20 = np.zeros((2, 5, 2, 128, 128), f32)
    linw20[:, :, 0] = lw[:, :, 0:128]
    linw20[:, :, 1, 0:8] = lw[:, :, 128:136]
    linw20 = linw20.reshape(20, 128, 128)
    A = g('lstm_wih').T                   # [256, 1024]
    B = g('lstm_whh').T
    lw32 = np.zeros((8, 4, 128, 128), f32)
    for s in range(8):
        for kc in range(4):
            src = A if kc < 2 else B
            k0 = (kc % 2) * 128
            lw32[s, kc] = src[k0:k0 + 128, s * 128:(s + 1) * 128]
    lw32 = lw32.reshape(32, 128, 128)

    biga = np.zeros((128, BIGA_W), BF_NP)
    biga[:, 0:128] = w1a
    biga[:, BIGA_W2:] = w2.transpose(1, 0, 2).reshape(128, 16 * 64)
    bigb = np.zeros((128, BIGB_W), BF_NP)
    bigb[:, 0:BIGB_LINW] = aw10.transpose(1, 0, 2).reshape(
        128, 10 * 64).astype(BF_NP)
    bigb[:, BIGB_LINW:BIGB_LW] = linw20.transpose(1, 0, 2).reshape(
        128, 20 * 128).astype(BF_NP)
    bigb[:, BIGB_LW:] = lw32.transpose(1, 0, 2).reshape(
        128, 32 * 128).astype(BF_NP)

    p64 = np.zeros((64, P64_W), BF_NP)
    p64[:, 0:N1] = x1b
    p64[:, P64_W3:P64_W1B] = w3.transpose(1, 0, 2).reshape(64, 16 * 64)
    p64[:, P64_W1B:P64_W1B + 128] = w1b
    p64[:, P64_ABT:] = abT.astype(BF_NP)

    # ---- f32 blobs ----
    f128 = np.zeros((128, 11), f32)
    f128[:, 0:2] = g('cx').reshape(2, 128).T
    headW = np.concatenate([g('critic_w'), g('actor_w')], axis=0).T  # [288,4]
    f128[:, 2:6] = headW[0:128]
    f128[:, 6:10] = headW[128:256]
    f128[:, 10] = g('conv1_b')
    f64 = np.zeros((64, 2), f32)
    f64[:, 0] = g('conv2_b')
    f64[:, 1] = g('conv3_b')
    f32s = np.zeros((32, 5), f32)
    f32s[:, 0] = g('time_emb')[tx]
    f32s[:, 1:5] = headW[256:288]
    f1 = np.zeros((1, 5), f32)
    f1[0, 0] = 1.0
    f1[0, 1] = g('critic_b').reshape(-1)[0]
    f1[0, 2:5] = g('actor_b').reshape(-1)
    return {
        "p33": p33, "gsm": gsm, "whh": whhb, "f128": f128, "x1a": x1a,
        "p64": p64, "biga": biga, "bigb": bigb, "f32s": f32s, "f64": f64,
        "f1": f1,
    }


def kernel(**inputs):
    if "nc" not in _CACHE:
        _CACHE["nc"] = _build()
    nc = _CACHE["nc"]
    in_map = _prep(inputs)
    in_maps = [in_map for _ in range(N_CORES)]
    res = run_bass_kernel_spmd(nc, in_maps, list(range(N_CORES)),
                               trace=_CACHE.get("trace", False))
    _CACHE["last"] = res
    out = np.asarray(res.results[0]["out"], dtype=np.float32)
    critic = out[0:1].reshape(1, 1)
    actor = out[1:4].reshape(1, 3)
    h2x = out[4:260].reshape(1, 256)
    c2 = out[260:516].reshape(1, 256)
    return critic, actor, h2x, c2


# revision 59
# speedup vs baseline: 1.0426x; 1.0005x over previous
"""A3C-LSTM-GA kernel for TRN2 (8 NeuronCores, SPMD-replicated).

Model: conv(8x8,s4) -> conv(4x4,s2) -> conv(4x4,s2) image backbone,
GRU(32->256) instruction encoder (18 sequential steps), 5-head gated
attention, linear(680->256), LSTMCell(256), critic/actor heads.

Strategy: batch-1 latency kernel; every core runs the identical program
(per the data-parallel sharding hint there is nothing to shard), output
taken from core 0. The 18-step GRU recurrence is the critical path; conv
work is scheduled by Tile into the PE/DVE idle slots of the GRU steps.

All matmuls run in bf16 (fp32 PSUM accumulation) except the tiny output
heads which stay fp32. All layout transforms / gathers / dtype casts of
constant inputs happen on the host inside kernel(); all FLOPs of the
model run on device.
"""

import numpy as np
import ml_dtypes

import concourse.bass as bass
import concourse.tile as tile
from concourse import bacc, mybir
from concourse.bass_utils import run_bass_kernel_spmd

BF = mybir.dt.bfloat16
F32 = mybir.dt.float32
AF = mybir.ActivationFunctionType
OP = mybir.AluOpType
BF_NP = ml_dtypes.bfloat16

N_CORES = 8

# ---- model dims (hardcoded; kernel.py must be self-contained) ----
IMG_H, IMG_W = 168, 300
OH1, OW1 = 41, 74      # conv1 out (stride 4, k8)
OH2, OW2 = 19, 36      # conv2 out (stride 2, k4)
OH3, OW3 = 8, 17       # conv3 out (stride 2, k4)
N1 = OH1 * OW1         # 3034
N2 = OH2 * OW2         # 684
N3 = OH3 * OW3         # 136
T = 18                 # instruction length
GH = 256               # GRU hidden

# blob layouts (element offsets into the packed DRAM parameters)
# p33 [33, 1042]: wordsb [33,18] | wih8 8x[33,128]
P33_W = 18 + 8 * 128
# gsm [128, 140]: ident [128,128] | hxc [128,2] | linb [128,2] | lb [128,8]
GSM_HXC = 128
GSM_LINB = GSM_HXC + 2
GSM_LB = GSM_LINB + 2
GSM_W = GSM_LB + 8
# whh [128, 1536]: whh12 12x[128,128]
# p64 [64, 4191]: x1b [64,3034] | w3 16x[64,64] | w1b [64,128] | abT [64,5]
P64_W3 = N1
P64_W1B = P64_W3 + 16 * 64
P64_ABT = P64_W1B + 128
P64_W = P64_ABT + 5
# biga [128, 1152]: w1a [128,128] | w2 16x[128,64]   (conv, needed early)
BIGA_W2 = 128
BIGA_W = BIGA_W2 + 16 * 64
# bigb [128, 7296]: aw10 10x[128,64] | linw 20x[128,128] | lw32 32x[128,128]
BIGB_LINW = 10 * 64
BIGB_LW = BIGB_LINW + 20 * 128
BIGB_W = BIGB_LW + 32 * 128
# f128 [128, 11] f32: cxc [128,2] | hdw [128,8] | b1 [128,1]
# f32s [32, 5] f32: temb [32,1] | hdw2 [32,4]
# f1  [1, 5] f32: onep [1,1] | hdb [1,4]

_CACHE = {}


def _build():
    """Build the Bass program once."""
    nc = bacc.Bacc("TRN2", target_bir_lowering=False, debug=False,
                   enable_asserts=False)

    d_p33 = nc.declare_dram_parameter("p33", [33, P33_W], BF, isOutput=False)
    d_gsm = nc.declare_dram_parameter("gsm", [128, GSM_W], BF, isOutput=False)
    d_whh = nc.declare_dram_parameter("whh", [128, 1536], BF, isOutput=False)
    d_f128 = nc.declare_dram_parameter("f128", [128, 11], F32, isOutput=False)
    d_x1a = nc.declare_dram_parameter("x1a", [128, N1], BF, isOutput=False)
    d_p64 = nc.declare_dram_parameter("p64", [64, P64_W], BF, isOutput=False)
    d_biga = nc.declare_dram_parameter("biga", [128, BIGA_W], BF, isOutput=False)
    d_bigb = nc.declare_dram_parameter("bigb", [128, BIGB_W], BF, isOutput=False)
    d_f32s = nc.declare_dram_parameter("f32s", [32, 5], F32, isOutput=False)
    d_f64 = nc.declare_dram_parameter("f64", [64, 2], F32, isOutput=False)
    d_f1 = nc.declare_dram_parameter("f1", [1, 5], F32, isOutput=False)
    d_out = nc.declare_dram_parameter("out", [516], F32, isOutput=True)

    with tile.TileContext(nc) as tc:
        with (
            tc.tile_pool(name="const", bufs=1) as cpool,
            tc.tile_pool(name="work", bufs=1) as wpool,
            tc.tile_pool(name="hpool", bufs=20) as hpool,
            tc.tile_pool(name="ew", bufs=20) as epool,
            tc.tile_pool(name="pstep", bufs=2, space="PSUM") as pstep,
            tc.tile_pool(name="pc1", bufs=2, space="PSUM") as pc1,
            tc.tile_pool(name="pc2", bufs=1, space="PSUM") as pc2,
            tc.tile_pool(name="ptail", bufs=1, space="PSUM") as ptail,
        ):
            # -------- input DMAs. GRU-critical blobs go FIRST on the sync
            # (HWDGE) queue in dependency order; only the big late-needed
            # weight blob rides gpsimd (SWDGE). The ACT engine issues no
            # DMAs so table loads + warmup run early.
            sp33 = cpool.tile([33, P33_W], BF)
            nc.sync.dma_start(out=sp33[:], in_=d_p33[:])
            sgsm = cpool.tile([128, GSM_W], BF)
            nc.sync.dma_start(out=sgsm[:], in_=d_gsm[:])
            swhh = cpool.tile([128, 1536], BF)
            nc.sync.dma_start(out=swhh[:], in_=d_whh[:])
            sx1a = cpool.tile([128, N1], BF)
            nc.sync.dma_start(out=sx1a[:], in_=d_x1a[:])
            sp64 = cpool.tile([64, P64_W], BF)
            nc.sync.dma_start(out=sp64[:], in_=d_p64[:])
            sf128 = cpool.tile([128, 11], F32)
            nc.sync.dma_start(out=sf128[:], in_=d_f128[:])
            sbiga = cpool.tile([128, BIGA_W], BF)
            nc.sync.dma_start(out=sbiga[:], in_=d_biga[:])
            sf64 = cpool.tile([64, 2], F32)
            nc.sync.dma_start(out=sf64[:], in_=d_f64[:])
            sf32s = cpool.tile([32, 5], F32)
            nc.sync.dma_start(out=sf32s[:], in_=d_f32s[:])
            sf1 = cpool.tile([1, 5], F32)
            nc.sync.dma_start(out=sf1[:], in_=d_f1[:])
            sbigb = cpool.tile([128, BIGB_W], BF)
            nc.sync.dma_start(out=sbigb[:], in_=d_bigb[:])

            ident = sgsm[:, 0:128]

            def whh_sl(kc, j):
                i = kc * 6 + j
                return swhh[:, i * 128:(i + 1) * 128]

            # ---------------- ACT table warmup ----------------------------
            # Trigger both table loads (sigmoid set, then the tanh set that
            # also holds sigmoid) during the DMA head, off the GRU path.
            warm = wpool.tile([1, 2], F32)
            nc.vector.memset(warm[:], 0.0)
            warm2 = wpool.tile([1, 2], F32)
            nc.scalar.activation(warm2[:], warm[:], AF.Sigmoid)
            nc.scalar.activation(warm2[:], warm[:], AF.Tanh)

            # ---------------- GRU input-side gates (GI) -------------------
            # psgi [128, 8, 18]; strip j: j<4 -> gi_rz + bih_rz + bhh_rz,
            # j in 4,5 -> bhh_g broadcast, j in 6,7 -> ig + bih_g
            psgi = pc2.tile([128, 8, T], F32, tag="c2")
            wordsb = sp33[:, 0:T]
            for j in range(8):
                wj = sp33[:, T + j * 128: T + (j + 1) * 128]
                nc.tensor.matmul(psgi[:, j, :], wj, wordsb,
                                 start=True, stop=True)
            gics = wpool.tile([128, 8, T], BF)
            nc.vector.tensor_copy(gics[:], psgi[:])

            # ---------------- GRU recurrence (critical path) --------------
            # rz gates and hg live in SEPARATE psum banks so the ACT sigmoid
            # read and the DVE hg evacuation don't serialize on one bank.
            h_prev = hpool.tile([128, 2], BF, tag="h")
            nc.vector.memset(h_prev[:], 0.0)

            for t in range(T):
                if t == 0:
                    # step 0: h=0, so the gates ARE the gi psum -- read it
                    # directly and skip the gics-copy dependency.
                    ps_rz = psgi[:, 0:4, 0]
                    ps_hg = psgi[:, 4:6, 0]
                else:
                    ps_rz_t = pstep.tile([128, 4], F32, tag="rz")
                    ps_hg_t = pstep.tile([128, 2], F32, tag="hg")
                    ps_rz = ps_rz_t[:]
                    ps_hg = ps_hg_t[:]
                    nc.tensor.matmul(ps_rz, ident, gics[:, 0:4, t],
                                     start=True, stop=False,
                                     skip_group_check=True)
                    for j in range(4):
                        for kc in range(2):
                            nc.tensor.matmul(
                                ps_rz[:, j:j + 1], whh_sl(kc, j),
                                h_prev[:, kc:kc + 1],
                                start=False, stop=(j == 3 and kc == 1),
                                skip_group_check=True)
                    nc.tensor.matmul(ps_hg, ident, gics[:, 4:6, t],
                                     start=True, stop=False,
                                     skip_group_check=True)
                    for j in (4, 5):
                        for kc in range(2):
                            nc.tensor.matmul(
                                ps_hg[:, j - 4:j - 3], whh_sl(kc, j),
                                h_prev[:, kc:kc + 1],
                                start=False, stop=(j == 5 and kc == 1),
                                skip_group_check=True)
                rz = epool.tile([128, 4], F32, tag="rz")
                nc.scalar.activation(rz[:], ps_rz, AF.Sigmoid)
                # off-critical-path (overlap the sigmoid):
                hgs = epool.tile([128, 2], F32, tag="hg")
                nc.vector.tensor_copy(hgs[:], ps_hg)
                # critical: u = r*hg ; v = u + ig ; g = tanh(v)
                ut = epool.tile([128, 2], F32, tag="u")
                nc.vector.tensor_mul(ut[:], rz[:, 0:2], hgs[:])
                vt = epool.tile([128, 2], F32, tag="v")
                nc.vector.tensor_add(vt[:], ut[:], gics[:, 6:8, t])
                gt = epool.tile([128, 2], F32, tag="g")
                nc.scalar.activation(gt[:], vt[:], AF.Tanh)
                # off-critical-path (overlap the tanh): p0 = z*h, w = 1-z
                p0 = epool.tile([128, 2], F32, tag="p0")
                nc.vector.tensor_mul(p0[:], rz[:, 2:4], h_prev[:])
                wt = epool.tile([128, 2], F32, tag="w")
                nc.vector.tensor_scalar(wt[:], rz[:, 2:4], -1.0, 1.0,
                                        OP.mult, OP.add)
                # critical tail: h' = g*w + p0
                r2 = epool.tile([128, 2], F32, tag="r2")
                nc.vector.tensor_mul(r2[:], gt[:], wt[:])
                h_new = hpool.tile([128, 2], BF, tag="h")
                nc.vector.tensor_add(h_new[:], r2[:], p0[:])
                h_prev = h_new

            # ---------------- conv1 --------------------------------------
            b1 = sf128[:, 10:11]
            h1 = wpool.tile([128, N1], BF)
            w1a = sbiga[:, 0:128]
            w1b = sp64[:, P64_W1B:P64_W1B + 128]
            x1b = sp64[:, 0:N1]
            # small matmul/relu chunks so conv work never blocks a GRU step's
            # PE/DVE ops for more than ~100-260 ns
            off = 0
            while off < N1:
                nw = min(256, N1 - off)
                ps1 = pc1.tile([128, nw], F32, tag="c1")
                nc.tensor.matmul(ps1[:], w1a, sx1a[:, off:off + nw],
                                 start=True, stop=False)
                nc.tensor.matmul(ps1[:], w1b[0:64, :], x1b[0:64, off:off + nw],
                                 start=False, stop=True)
                roff = 0
                while roff < nw:
                    rw = min(128, nw - roff)
                    nc.vector.tensor_scalar(
                        h1[:, off + roff:off + roff + rw],
                        ps1[:, roff:roff + rw], b1, 0.0, OP.add, OP.max)
                    roff += rw
                off += nw

            # ---------------- conv2 --------------------------------------
            h1v = h1[:].rearrange("p (h w) -> p h w", h=OH1, w=OW1)
            h2 = wpool.tile([64, N2], BF)
            h2v = h2[:].rearrange("p (h w) -> p h w", h=OH2, w=OW2)
            for half in range(2):
                ps2 = pc2.tile([64, OH2 * 18], F32, tag="c2")
                for c16 in range(16):
                    kh, kw = divmod(c16, 4)
                    rhs = h1v[:, kh:kh + 37:2,
                              kw + 36 * half:kw + 36 * half + 35:2]
                    nc.tensor.matmul(ps2[:], sbiga[:, BIGA_W2 + c16 * 64:
                                                    BIGA_W2 + (c16 + 1) * 64],
                                     rhs, start=(c16 == 0), stop=(c16 == 15))
                ps2v = ps2[:].rearrange("p (h w) -> p h w", h=OH2, w=18)
                for rh in range(2):
                    nc.vector.tensor_scalar(
                        h2v[:, :, 18 * half + 9 * rh:18 * half + 9 * rh + 9],
                        ps2v[:, :, 9 * rh:9 * rh + 9],
                        sf64[:, 0:1], 0.0, OP.add, OP.max)
            # ---------------- conv3 --------------------------------------
            img = wpool.tile([64, N3], BF)
            ps3 = ptail.tile([64, N3], F32, tag="tail")
            for c16 in range(16):
                kh, kw = divmod(c16, 4)
                rhs = h2v[:, kh:kh + 15:2, kw:kw + 33:2]
                nc.tensor.matmul(ps3[:], sp64[0:64, P64_W3 + c16 * 64:
                                              P64_W3 + (c16 + 1) * 64],
                                 rhs, start=(c16 == 0), stop=(c16 == 15))
            nc.vector.tensor_scalar(img[:], ps3[:],
                                    sf64[:, 1:2], 0.0, OP.add, OP.max)

            # ---------------- gated attention -----------------------------
            instr = h_prev  # [128, 2] bf16
            psA = ptail.tile([64, 5], F32, tag="tail")
            nc.tensor.matmul(psA[:], ident[0:64, 0:64],
                             sp64[0:64, P64_ABT:P64_ABT + 5],
                             start=True, stop=False, skip_group_check=True)
            for kk in range(5):
                for kc in range(2):
                    i = kk * 2 + kc
                    nc.tensor.matmul(psA[:, kk:kk + 1],
                                     sbigb[:, i * 64:(i + 1) * 64],
                                     instr[:, kc:kc + 1],
                                     start=False, stop=(kk == 4 and kc == 1),
                                     skip_group_check=True)
            attn = wpool.tile([64, 5], BF)
            nc.vector.tensor_scalar_max(attn[:], psA[:], 0.0)

            # attn_vecs TRANSPOSED directly: avT = img.T @ attn, so the
            # linear layer's lhsT columns come out of PSUM with no PE
            # transposes and no extra copies. (Reuses idle GRU psum slots.)
            psTa = pstep.tile([128, 5], F32, tag="rz")
            nc.tensor.matmul(psTa[:], img[:, 0:128], attn[:],
                             start=True, stop=True)
            psTb = pstep.tile([8, 5], F32, tag="hg")
            nc.tensor.matmul(psTb[:], img[:, 128:136], attn[:],
                             start=True, stop=True)
            avTa = wpool.tile([128, 5], BF)
            nc.vector.tensor_copy(avTa[:], psTa[:])
            avTb = wpool.tile([8, 5], BF)
            nc.vector.tensor_copy(avTb[:], psTb[:])

            # ---------------- linear 680 -> 256 ---------------------------
            psF = pc2.tile([128, 2], F32, tag="c2")
            nc.tensor.matmul(psF[:], ident, sgsm[:, GSM_LINB:GSM_LINB + 2],
                             start=True, stop=False, skip_group_check=True)
            for c in range(2):
                for kk in range(5):
                    i = (c * 5 + kk) * 2
                    nc.tensor.matmul(psF[:, c:c + 1],
                                     sbigb[:, BIGB_LINW + i * 128:
                                          BIGB_LINW + (i + 1) * 128],
                                     avTa[:, kk:kk + 1],
                                     start=False, stop=False,
                                     skip_group_check=True)
                    sl = sbigb[:, BIGB_LINW + (i + 1) * 128:
                               BIGB_LINW + (i + 2) * 128]
                    nc.tensor.matmul(psF[:, c:c + 1], sl[0:8, :],
                                     avTb[:, kk:kk + 1],
                                     start=False, stop=(kk == 4),
                                     skip_group_check=True)
            feat = wpool.tile([128, 2], BF)
            nc.vector.tensor_scalar_max(feat[:], psF[:], 0.0)

            # ---------------- LSTM cell -----------------------------------
            # bias + hx-side matmuls have no feat dependency -- listed first
            # so they run during the attention/linear phase.
            hxc = sgsm[:, GSM_HXC:GSM_HXC + 2]
            cxc = sf128[:, 0:2]
            psL = pc1.tile([128, 8], F32, tag="c1")
            nc.tensor.matmul(psL[:], ident, sgsm[:, GSM_LB:GSM_LB + 8],
                             start=True, stop=False, skip_group_check=True)
            for kc in (2, 3):
                for s in range(8):
                    i = s * 4 + kc
                    nc.tensor.matmul(psL[:, s:s + 1],
                                     sbigb[:, BIGB_LW + i * 128:
                                          BIGB_LW + (i + 1) * 128],
                                     hxc[:, kc - 2:kc - 1], start=False,
                                     stop=False, skip_group_check=True)
            for kc in (0, 1):
                for s in range(8):
                    i = s * 4 + kc
                    nc.tensor.matmul(psL[:, s:s + 1],
                                     sbigb[:, BIGB_LW + i * 128:
                                          BIGB_LW + (i + 1) * 128],
                                     feat[:, kc:kc + 1], start=False,
                                     stop=(s == 7 and kc == 1),
                                     skip_group_check=True)
            sg = epool.tile([128, 8], F32, tag="sg")
            nc.scalar.activation(sg[:], psL[:], AF.Sigmoid)
            tg = epool.tile([128, 2], F32, tag="tg")
            nc.scalar.activation(tg[:], psL[:, 4:6], AF.Tanh)
            # heads rhs folding: (hdw * sigma_o) so the head matmuls need
            # only tanh(c2), shaving the final h2 multiply off their path
            hdp0 = wpool.tile([128, 4], F32)
            nc.vector.tensor_scalar(hdp0[:], sf128[:, 2:6], sg[:, 6:7],
                                    None, OP.mult)
            hdp1 = wpool.tile([128, 4], F32)
            nc.vector.tensor_scalar(hdp1[:], sf128[:, 6:10], sg[:, 7:8],
                                    None, OP.mult)
            m1 = epool.tile([128, 2], F32, tag="m1")
            nc.vector.tensor_mul(m1[:], sg[:, 2:4], cxc)
            m2 = epool.tile([128, 2], F32, tag="m2")
            nc.vector.tensor_mul(m2[:], sg[:, 0:2], tg[:])
            hc2 = wpool.tile([128, 4], F32)   # cols 0:2 h2x, 2:4 c2
            c2o = hc2[:, 2:4]
            nc.vector.tensor_add(c2o, m1[:], m2[:])
            tc2 = epool.tile([128, 2], F32, tag="tc2")
            nc.scalar.activation(tc2[:], c2o, AF.Tanh)
            h2o = hc2[:, 0:2]
            nc.vector.tensor_mul(h2o, sg[:, 6:8], tc2[:])

            # ---------------- heads (fp32) --------------------------------
            psH = pc1.tile([1, 4], F32, tag="c1")
            nc.tensor.matmul(psH[:], sf1[:, 0:1], sf1[:, 1:5],
                             start=True, stop=False, skip_group_check=True)
            nc.tensor.matmul(psH[:], sf32s[:, 0:1], sf32s[:, 1:5],
                             start=False, stop=False, skip_group_check=True)
            nc.tensor.matmul(psH[:], tc2[:, 0:1], hdp0[:],
                             start=False, stop=False, skip_group_check=True)
            nc.tensor.matmul(psH[:], tc2[:, 1:2], hdp1[:],
                             start=False, stop=True, skip_group_check=True)

            oH = wpool.tile([1, 4], F32)
            nc.vector.tensor_copy(oH[:], psH[:])

            # ---------------- outputs (2 parallel DMAs) -------------------
            nc.sync.dma_start(out=d_out[0:4], in_=oH[:])
            ohc = d_out[4:516].rearrange("(c p) -> p c", p=128)
            nc.sync.dma_start(out=ohc, in_=hc2[:])

    nc.compile()
    return nc


def _prep(inputs):
    """Host-side layout prep. Returns the in_map for one core."""
    f32 = np.float32
    g = lambda k: np.asarray(inputs[k], dtype=f32)
    x = g('x')[0]                         # [3, 168, 300]
    input_inst = np.asarray(inputs['input_inst']).astype(np.int64)[0]
    tx = int(np.asarray(inputs['tx']).reshape(-1)[0])

    # ---- conv1 im2col [192, 3034] ----
    sc, sh, sw = x.strides
    pat = np.lib.stride_tricks.as_strided(
        x, shape=(3, 8, 8, OH1, OW1), strides=(sc, sh, sw, 4 * sh, 4 * sw))
    X = pat.reshape(192, N1)
    x1a = np.ascontiguousarray(X[0:128]).astype(BF_NP)
    x1b = X[128:192].astype(BF_NP)        # [64, 3034]

    w1 = g('conv1_w').transpose(1, 2, 3, 0).reshape(192, 128)  # [k, m]
    w1a = w1[0:128].astype(BF_NP)
    w1b = w1[128:192].astype(BF_NP)       # [64, 128]
    w2 = g('conv2_w').transpose(2, 3, 1, 0).reshape(16, 128, 64).astype(BF_NP)
    w3 = g('conv3_w').transpose(2, 3, 1, 0).reshape(16, 64, 64).astype(BF_NP)

    # ---- GRU prep ----
    emb = g('emb')
    words = emb[input_inst]               # [18, 32]
    wordsb = np.ones((33, T), f32)
    wordsb[0:32] = words.T
    wih = g('gru_wih')                    # [768, 32]
    bih = g('gru_bih')
    bhh = g('gru_bhh')
    wih8 = np.zeros((8, 33, 128), f32)
    for j in range(4):                    # rz strips + both biases
        wih8[j, 0:32] = wih[j * 128:(j + 1) * 128].T
        wih8[j, 32] = bih[j * 128:(j + 1) * 128] + bhh[j * 128:(j + 1) * 128]
    for j in (4, 5):                      # bhh_g broadcast strips
        wih8[j, 32] = bhh[j * 128:(j + 1) * 128]
    for j in (6, 7):                      # true ig strips
        wih8[j, 0:32] = wih[(j - 2) * 128:(j - 1) * 128].T
        wih8[j, 32] = bih[(j - 2) * 128:(j - 1) * 128]
    p33 = np.zeros((33, P33_W), BF_NP)
    p33[:, 0:T] = wordsb.astype(BF_NP)
    p33[:, T:] = wih8.transpose(1, 0, 2).reshape(33, 8 * 128).astype(BF_NP)

    whh = g('gru_whh')                    # [768, 256]
    whh12 = whh.T.reshape(2, 128, 6, 128).transpose(0, 2, 1, 3)  # kc,j,k,m
    hx = g('hx').reshape(2, 128).T        # [128, 2]
    lin_b = g('lin_b').reshape(2, 128).T  # [128, 2]
    lstm_b = (g('lstm_bih') + g('lstm_bhh')).reshape(8, 128).T   # [128, 8]
    whhb = whh12.reshape(12, 128, 128).transpose(1, 0, 2).reshape(
        128, 12 * 128).astype(BF_NP)
    gsm = np.zeros((128, GSM_W), BF_NP)
    gsm[:, 0:128] = np.eye(128, dtype=BF_NP)
    gsm[:, GSM_HXC:GSM_HXC + 2] = hx.astype(BF_NP)
    gsm[:, GSM_LINB:GSM_LINB + 2] = lin_b.astype(BF_NP)
    gsm[:, GSM_LB:GSM_LB + 8] = lstm_b.astype(BF_NP)

    # ---- attention / linear / lstm weights ----
    aw = g('attn_w').transpose(0, 2, 1).reshape(5, 2, 128, 64)  # kk,kc,k,c
    aw10 = aw.reshape(10, 128, 64)
    abT = g('attn_b').T                   # [64, 5]
    lin_w = g('lin_w')                    # [256, 680]
    lw = lin_w.reshape(2, 128, 5, 136).transpose(0, 2, 3, 1)  # c,kk,hw,m
    linw20 = np.zeros((2, 5, 2, 128, 128), f32)
    linw20[:, :, 0] = lw[:, :, 0:128]
    linw20[:, :, 1, 0:8] = lw[:, :, 128:136]
    linw20 = linw20.reshape(20, 128, 128)
    A = g('lstm_wih').T                   # [256, 1024]
    B = g('lstm_whh').T
    lw32 = np.zeros((8, 4, 128, 128), f32)
    for s in range(8):
        for kc in range(4):
            src = A if kc < 2 else B
            k0 = (kc % 2) * 128
            lw32[s, kc] = src[k0:k0 + 128, s * 128:(s + 1) * 128]
    lw32 = lw32.reshape(32, 128, 128)

    biga = np.zeros((128, BIGA_W), BF_NP)
    biga[:, 0:128] = w1a
    biga[:, BIGA_W2:] = w2.transpose(1, 0, 2).reshape(128, 16 * 64)
    bigb = np.zeros((128, BIGB_W), BF_NP)
    bigb[:, 0:BIGB_LINW] = aw10.transpose(1, 0, 2).reshape(
        128, 10 * 64).astype(BF_NP)
    bigb[:, BIGB_LINW:BIGB_LW] = linw20.transpose(1, 0, 2).reshape(
        128, 20 * 128).astype(BF_NP)
    bigb[:, BIGB_LW:] = lw32.transpose(1, 0, 2).reshape(
        128, 32 * 128).astype(BF_NP)

    p64 = np.zeros((64, P64_W), BF_NP)
    p64[:, 0:N1] = x1b
    p64[:, P64_W3:P64_W1B] = w3.transpose(1, 0, 2).reshape(64, 16 * 64)
    p64[:, P64_W1B:P64_W1B + 128] = w1b
    p64[:, P64_ABT:] = abT.astype(BF_NP)

    # ---- f32 blobs ----
    f128 = np.zeros((128, 11), f32)
    f128[:, 0:2] = g('cx').reshape(2, 128).T
    headW = np.concatenate([g('critic_w'), g('actor_w')], axis=0).T  # [288,4]
    f128[:, 2:6] = headW[0:128]
    f128[:, 6:10] = headW[128:256]
    f128[:, 10] = g('conv1_b')
    f64 = np.zeros((64, 2), f32)
    f64[:, 0] = g('conv2_b')
    f64[:, 1] = g('conv3_b')
    f32s = np.zeros((32, 5), f32)
    f32s[:, 0] = g('time_emb')[tx]
    f32s[:, 1:5] = headW[256:288]
    f1 = np.zeros((1, 5), f32)
    f1[0, 0] = 1.0
    f1[0, 1] = g('critic_b').reshape(-1)[0]
    f1[0, 2:5] = g('actor_b').reshape(-1)
    return {
        "p33": p33, "gsm": gsm, "whh": whhb, "f128": f128, "x1a": x1a,
        "p64": p64, "biga": biga, "bigb": bigb, "f32s": f32s, "f64": f64,
        "f1": f1,
    }


def kernel(**inputs):
    if "nc" not in _CACHE:
        _CACHE["nc"] = _build()
    nc = _CACHE["nc"]
    in_map = _prep(inputs)
    in_maps = [in_map for _ in range(N_CORES)]
    res = run_bass_kernel_spmd(nc, in_maps, list(range(N_CORES)),
                               trace=_CACHE.get("trace", False))
    _CACHE["last"] = res
    out = np.asarray(res.results[0]["out"], dtype=np.float32)
    critic = out[0:1].reshape(1, 1)
    actor = out[1:4].reshape(1, 3)
    h2x = out[4:260].reshape(1, 256)
    c2 = out[260:516].reshape(1, 256)
    return critic, actor, h2x, c2
